# revision 13
# baseline (speedup 1.0000x reference)
"""Multi-head self-attention (B=4, S=2048, E=1024, H=16, D=64) on 8 TRN2 cores.

Sharding: core c handles batch b = c//2 and head-group hg = c%2 (8 of 16 heads).
QKV weights column-parallel, proj row-parallel (Megatron); the two cores
sharing a batch produce partial proj outputs that are summed on the host.

Device layout (per core):
  xt = x[b].T [E, S] resident in SBUF; qT/kT [512 feats, 2048] with head
  pair p packing head 2p on partitions 0:64 and head 2p+1 on 64:128, so a
  pair's two K=64 score matmuls run concurrently on disjoint PE row groups
  (the per-chunk rate is then set by the weight-load port: ~2x128 columns
  per 128-k-chunk, balancing the concurrent N=512 streams).
  Scores for each (k-chunk, head) land in one PSUM-bank stage slot; stages
  ping-pong between a 3-slot and a 2-slot tile so the softmax exp runs as
  1536/1024-wide scalar-engine calls into a rolling fp16 pT buffer.
  Softmax without max-subtraction (exp(s/8 - 4) is fp16-safe); the
  denominator rides the AV matmul as a ones-augmented [V | 1] stationary
  (M=65); AV accumulates [65, 2 heads, 512 q] per block in two PSUM banks.
  Normalization: PE broadcast of the den row, vector reciprocal + multiply
  into fp16 attnT; proj accumulated from attnT in fp16, fp16 output.
"""

import numpy as np

B, S, E = 4, 2048, 1024
H, D = 16, 64
HLOC = 8          # heads per core
FEAT = HLOC * D   # 512 per-core q/k/v features
NCORES = 8
NP = HLOC // 2    # 4 head pairs
KC = S // 128     # 16 k-chunks
QC = S // 512     # 4 q-blocks per pair
CH = 512
NCH = S // CH     # 4 x-chunks
EC = E // 128     # 8 e-chunks
NSLOT = 15        # pT rolling slots (multiple of 5 = s1+s2 group period)

_CACHE = {}


def _build_program():
    import concourse.bass as bass
    import concourse.mybir as mybir
    from concourse import bacc
    from concourse.tile import TileContext
    from contextlib import ExitStack
    from collections import deque

    F32R = mybir.dt.float32r
    F32 = mybir.dt.float32
    F16 = mybir.dt.float16
    AF = mybir.ActivationFunctionType

    nc = bacc.Bacc("TRN2", target_bir_lowering=False, num_devices=NCORES)

    xt = nc.dram_tensor("xt", [E, S], F16, kind="ExternalInput")
    wq = nc.dram_tensor("wq", [E, FEAT], F16, kind="ExternalInput")
    wk = nc.dram_tensor("wk", [E, FEAT], F16, kind="ExternalInput")
    wv = nc.dram_tensor("wv", [E, FEAT], F16, kind="ExternalInput")
    wp = nc.dram_tensor("wp", [FEAT, E], F16, kind="ExternalInput")
    out = nc.dram_tensor("out", [S, E], F16, kind="ExternalOutput")

    xt_v = xt.ap().rearrange("(c p) s -> p c s", p=128)
    wq_v = wq.ap().rearrange("(c p) f -> p c f", p=128)
    wk_v = wk.ap().rearrange("(c p) f -> p c f", p=128)
    wv_v = wv.ap().rearrange("(c p) f -> p c f", p=128)
    wp_v = wp.ap().rearrange("(c p) n -> p c n", p=128)

    with TileContext(nc) as tc:
      with ExitStack() as es:
        from collections import deque as _dq

        pp = es.enter_context(tc.tile_pool(name="persist", bufs=1))
        qT = pp.tile([128, NP, S], F16)
        kT = pp.tile([128, NP, S], F16)
        v1 = pp.tile([128, KC, HLOC, D + 1], F16)
        attnT = pp.tile([128, NP, S], F16)
        pT = pp.tile([128, NSLOT, 512], F16)
        xt_sb = pp.tile([128, EC, S], F16)
        wq_sb = pp.tile([128, EC, FEAT], F16)
        wk_sb = pp.tile([128, EC, FEAT], F16)
        wv_sb = pp.tile([128, EC, FEAT], F16)
        wp_sb = pp.tile([128, NP, E], F16)
        ones1 = pp.tile([1, 128], F32R)
        neg4 = pp.tile([128, 1], F32)

        pnm = es.enter_context(tc.tile_pool(name="pnorm", bufs=2))
        pout = es.enter_context(tc.tile_pool(name="pout", bufs=3))
        pstg = es.enter_context(tc.tile_pool(name="pstg", bufs=1, space="PSUM"))
        pso = es.enter_context(tc.tile_pool(name="pso", bufs=1, space="PSUM"))
        paux = es.enter_context(tc.tile_pool(name="paux", bufs=1, space="PSUM"))

        s1 = pstg.tile([128, 3, 512], F32)   # 3 banks: 1536-wide exp
        s2 = pstg.tile([128, 2, 512], F32)   # 2 banks: 1024-wide exp

        nc.gpsimd.memset(neg4[:], -4.0)

        # ---- DMA priority order: pair-0 weights + x chunk 0 first ----
        nc.sync.dma_start(wk_sb[:, :, 0:128], wk_v[:, :, 0:128])
        nc.sync.dma_start(wq_sb[:, :, 0:128], wq_v[:, :, 0:128])
        nc.sync.dma_start(xt_sb[:, 0:EC // 2, 0:CH], xt_v[:, 0:EC // 2, 0:CH])
        nc.sync.dma_start(xt_sb[:, EC // 2:, 0:CH], xt_v[:, EC // 2:, 0:CH])
        # constants: ones row + the ones column of [V | 1] (ACT const fill)
        nc.scalar.activation(ones1[:], wk_sb[0:1, 0, 0:128],
                             AF.Copy, bias=1.0, scale=0.0)
        nc.scalar.activation(
            v1[:, :, :, D],
            wk_sb[:, 0, 0:KC * HLOC].rearrange("p (a b) -> p a b", a=KC),
            AF.Copy, bias=1.0, scale=0.0)
        nc.sync.dma_start(wv_sb[:, 0:EC // 2], wv_v[:, 0:EC // 2])
        nc.sync.dma_start(wv_sb[:, EC // 2:], wv_v[:, EC // 2:])
        for ch in range(1, NCH):
            csl = slice(ch * CH, (ch + 1) * CH)
            nc.sync.dma_start(xt_sb[:, 0:EC // 2, csl],
                              xt_v[:, 0:EC // 2, csl])
            nc.sync.dma_start(xt_sb[:, EC // 2:, csl], xt_v[:, EC // 2:, csl])
        nc.sync.dma_start(wk_sb[:, :, 128:FEAT], wk_v[:, :, 128:FEAT])
        nc.sync.dma_start(wq_sb[:, :, 128:FEAT], wq_v[:, :, 128:FEAT])
        nc.sync.dma_start(wp_sb[:], wp_v)

        # ---------- aux-psum unit builders ----------
        # Units are ATOMIC: each emits its full matmul group + evacuation
        # before returning, so the single aux PSUM bank never has an open
        # accumulation group when another user allocates it (a mid-group
        # interleave would create a PE<->DVE FIFO deadlock).
        def qk_unit(dst, w_sb, fc, ch):
            """Q/K projection for one (pair, x-chunk)."""
            ps1 = paux.tile([128, 512], F32, tag="aux", name="ps1")
            csl = slice(ch * CH, (ch + 1) * CH)
            for ec in range(EC):
                nc.tensor.matmul(
                    ps1[:], w_sb[:, ec, fc * 128:(fc + 1) * 128],
                    xt_sb[:, ec, csl],
                    start=(ec == 0), stop=(ec == EC - 1))
            with nc.allow_low_precision(reason="fp16 attn"):
                nc.vector.tensor_copy(dst[:, fc, csl], ps1[:])

        def v_unit(kcg):
            """V projection for one 128-row k-chunk (all 8 heads)."""
            ps1 = paux.tile([128, 512], F32, tag="aux", name="ps1")
            ssl = slice(kcg * 128, (kcg + 1) * 128)
            for ec in range(EC):
                nc.tensor.matmul(ps1[:], xt_sb[:, ec, ssl], wv_sb[:, ec, :],
                                 start=(ec == 0), stop=(ec == EC - 1))
            with nc.allow_low_precision(reason="fp16 attn"):
                nc.vector.tensor_copy(
                    v1[:, kcg, :, 0:D],
                    ps1.rearrange("p (h d) -> p h d", h=HLOC))

        def proj_unit(sc, n2):
            """out[sc*128:+128, n2-half] = attnT.T @ wp (one 512-col half)."""
            ssl = slice(sc * 128, (sc + 1) * 128)
            nsl = slice(n2 * 512, (n2 + 1) * 512)
            ps_p = paux.tile([128, 512], F32, tag="aux", name="ps_p")
            for fc in range(NP):
                nc.tensor.matmul(ps_p[:], attnT[:, fc, ssl],
                                 wp_sb[:, fc, nsl],
                                 start=(fc == 0), stop=(fc == NP - 1))
            out_t = pout.tile([128, 512], F16, tag="out", name="out_t")
            with nc.allow_low_precision(reason="fp16 attn"):
                nc.vector.tensor_copy(out_t[:], ps_p[:])
            nc.sync.dma_start(out.ap()[ssl, nsl], out_t[:])

        work = _dq()

        def pull(n):
            while n > 0 and work:
                work.popleft()()
                n -= 1

        # ---------- attention emitter ----------
        cur_o = {}
        av_emitted = set()          # blocks whose kc==15 AV has been emitted
        g_slot = [0]                # next free pT slot (wraps by group)
        grp = {"tiles": [], "kind": 0}   # kind 0 -> s1 (3 slots), 1 -> s2 (2)
        av_levels = _dq()
        slot_of = {}                # (p, qc, kc) -> head-A pT slot
        pending_norms = _dq()       # (block, norm_fn)

        def av_pair(p, qc, kc, slot_a, slot_b):
            o = cur_o[(p, qc)]
            st, sp = (kc == 0), (kc == KC - 1)
            nc.tensor.matmul(o[0:D + 1, 0, :], v1[:, kc, 2 * p, :],
                             pT[:, slot_a, :], start=st, stop=sp)
            nc.tensor.matmul(o[0:D + 1, 1, :], v1[:, kc, 2 * p + 1, :],
                             pT[:, slot_b, :], start=st, stop=sp)
            if sp:
                av_emitted.add((p, qc))

        def drain_level():
            for p, qc, kc, head, slot in av_levels.popleft():
                if head == 0:
                    slot_of[(p, qc, kc)] = slot
                else:
                    av_pair(p, qc, kc, slot_of.pop((p, qc, kc)), slot)

        def flush_group():
            tiles = grp["tiles"]
            if not tiles:
                return
            st = s1 if grp["kind"] == 0 else s2
            n = len(tiles)
            if g_slot[0] + n > NSLOT:
                g_slot[0] = 0
            base = g_slot[0]
            g_slot[0] += n
            with nc.allow_low_precision(reason="fp16 attn"):
                nc.scalar.activation(pT[:, base:base + n, :], st[:, 0:n, :],
                                     AF.Exp, scale=0.125, bias=neg4[:])
            av_levels.append([(p, qc, kc, head, base + i)
                              for i, (p, qc, kc, head) in enumerate(tiles)])
            grp["tiles"] = []
            grp["kind"] ^= 1
            # emit AV lagged 2 exp-groups so the PE never waits on ACT
            while len(av_levels) > 2:
                drain_level()

        def emit_scores(p, qc, kc):
            """One k-chunk of scores for both heads of a pair: two slots."""
            ksl = slice(kc * 128, (kc + 1) * 128)
            qsl = slice(qc * 512, (qc + 1) * 512)
            for head in range(2):
                st = s1 if grp["kind"] == 0 else s2
                j = len(grp["tiles"])
                rows = slice(64 * head, 64 * head + 64)
                nc.tensor.matmul(st[:, j, :], kT[rows, p, ksl],
                                 qT[rows, p, qsl], start=True, stop=True)
                grp["tiles"].append((p, qc, kc, head))
                cap = 3 if grp["kind"] == 0 else 2
                if len(grp["tiles"]) == cap:
                    flush_group()

        def make_norm(p, qc, o):
            def norm():
                den = pnm.tile([1, 2, 512], F32R, tag="den", name="den")
                with nc.allow_low_precision(reason="fp16 attn"):
                    nc.vector.tensor_copy(den[:], o[D:D + 1, :, :])
                for h in range(2):
                    ps_b = paux.tile([128, 512], F32, tag="aux", name="ps_b")
                    nc.tensor.matmul(ps_b[0:64, :], ones1[:, 0:64],
                                     den[:, h, :], start=True, stop=True)
                    r_sb = pnm.tile([64, 512], F32, tag=f"r{h}",
                                    name="r_sb")
                    nc.vector.reciprocal_approx_fast(out=r_sb[:],
                                                     in_=ps_b[0:64, :])
                    qsl = slice(qc * 512, (qc + 1) * 512)
                    with nc.allow_low_precision(reason="fp16 attn"):
                        nc.vector.tensor_mul(attnT[64 * h:64 * h + 64, p, qsl],
                                             o[0:D, h, :], r_sb[:])
                if p == NP - 1:
                    for sc in range(4 * qc, 4 * qc + 4):
                        for n2 in range(2):
                            work.append(
                                lambda sc=sc, n2=n2: proj_unit(sc, n2))
            return norm

        def run_norm_front():
            b, fn = pending_norms[0]
            while b not in av_emitted:
                if not av_levels:
                    flush_group()
                else:
                    drain_level()
            pending_norms.popleft()
            fn()

        # ---------- pass 1: pair-0 q/k, all v, block (0,0) ----------
        cur_o[(0, 0)] = pso.tile([D + 1, 2, 512], F32, tag="o", name="ps_o")
        for ch in range(NCH):
            qk_unit(kT, wk_sb, 0, ch)
            qk_unit(qT, wq_sb, 0, ch)
            for sc2 in range(4):
                v_unit(4 * ch + sc2)
            for kcg in range(4 * ch, 4 * ch + 4):
                emit_scores(0, 0, kcg)
        pending_norms.append(((0, 0), make_norm(0, 0, cur_o[(0, 0)])))

        # ---------- pass 2: remaining blocks, qk pairs 1-3 via pulls ----
        for fc in range(1, NP):
            for ch in range(NCH):
                work.append(lambda f=fc, c=ch: qk_unit(kT, wk_sb, f, c))
                work.append(lambda f=fc, c=ch: qk_unit(qT, wq_sb, f, c))

        blocks = [(0, qc) for qc in range(1, QC)]
        blocks += [(p, qc) for p in range(1, NP) for qc in range(QC)]
        for p, qc in blocks:
            while pending_norms:
                run_norm_front()
            o = pso.tile([D + 1, 2, 512], F32, tag="o", name="ps_o")
            cur_o[(p, qc)] = o
            for kc in range(KC):
                emit_scores(p, qc, kc)
                if (p == NP - 1) or (kc % 3 == 2):
                    pull(1)
            pending_norms.append(((p, qc), make_norm(p, qc, o)))

        while pending_norms:
            run_norm_front()
        while av_levels:
            drain_level()
        pull(10 ** 9)

    nc.compile()
    return nc


def _prep_inputs(x, W_qkv, W_proj):
    """Build the 8 per-core input maps (host-side sharding/layout only)."""
    Wr = np.ascontiguousarray(W_qkv.reshape(E, 3, H, D))
    in_maps = []
    for c in range(NCORES):
        b, hg = c // 2, c % 2
        hsl = slice(hg * HLOC, (hg + 1) * HLOC)
        m = {
            "xt": np.ascontiguousarray(x[b].T).astype(np.float16),
            "wq": np.ascontiguousarray(
                Wr[:, 0, hsl, :].reshape(E, FEAT)).astype(np.float16),
            "wk": np.ascontiguousarray(
                Wr[:, 1, hsl, :].reshape(E, FEAT)).astype(np.float16),
            "wv": np.ascontiguousarray(
                Wr[:, 2, hsl, :].reshape(E, FEAT)).astype(np.float16),
            "wp": np.ascontiguousarray(
                W_proj[hg * FEAT:(hg + 1) * FEAT, :]).astype(np.float16),
        }
        in_maps.append(m)
    return in_maps


def _run_fallback(x, W_qkv, b_qkv, W_proj, b_proj):
    """Host-side reference path (only used when biases are nonzero)."""
    scale = 1.0 / np.sqrt(D)
    out = np.empty((B, S, E), dtype=np.float32)
    qkv = (x.reshape(B * S, E) @ W_qkv + b_qkv).reshape(B, S, 3, H, D)
    q, k, v = qkv[:, :, 0], qkv[:, :, 1], qkv[:, :, 2]
    for b in range(B):
        ob = np.empty((S, E), np.float32)
        for h in range(H):
            s = (q[b, :, h] @ k[b, :, h].T) * scale
            s -= s.max(axis=1, keepdims=True)
            p = np.exp(s)
            p /= p.sum(axis=1, keepdims=True)
            ob[:, h * D:(h + 1) * D] = p @ v[b, :, h]
        out[b] = ob @ W_proj + b_proj
    return out


def run(x, W_qkv, b_qkv, W_proj, b_proj, trace=False):
    from concourse.bass_utils import run_bass_kernel_spmd

    if bool(np.any(b_qkv)) or bool(np.any(b_proj)):
        return _run_fallback(x, W_qkv, b_qkv, W_proj, b_proj), None

    if "nc" not in _CACHE:
        _CACHE["nc"] = _build_program()
    nc = _CACHE["nc"]

    in_maps = _prep_inputs(x, W_qkv, W_proj)
    res = run_bass_kernel_spmd(nc, in_maps, core_ids=list(range(NCORES)),
                               trace=trace)
    out = np.empty((B, S, E), dtype=np.float32)
    for b in range(B):
        out[b] = (res.results[2 * b]["out"].astype(np.float32)
                  + res.results[2 * b + 1]["out"].astype(np.float32))
    return out, res


def kernel(x, W_qkv, b_qkv, W_proj, b_proj):
    out, _ = run(np.asarray(x), np.asarray(W_qkv), np.asarray(b_qkv),
                 np.asarray(W_proj), np.asarray(b_proj))
    return out


# revision 19
# speedup vs baseline: 1.0594x; 1.0594x over previous
"""Multi-head self-attention (B=4, S=2048, E=1024, H=16, D=64) on 8 TRN2 cores.

Sharding: core c handles batch b = c//2 and head-group hg = c%2 (8 of 16 heads).
QKV weights column-parallel, proj row-parallel (Megatron); the two cores
sharing a batch produce partial proj outputs that are summed on the host.

Device layout (per core):
  xt = x[b].T [E, S] resident in SBUF; qT/kT [512 feats, 2048] with head
  pair p packing head 2p on partitions 0:64 and head 2p+1 on 64:128, so a
  pair's two K=64 score matmuls run concurrently on disjoint PE row groups
  (the per-chunk rate is then set by the weight-load port: ~2x128 columns
  per 128-k-chunk, balancing the concurrent N=512 streams).
  Scores for each (k-chunk, head) land in one PSUM-bank stage slot; stages
  ping-pong between a 3-slot and a 2-slot tile so the softmax exp runs as
  1536/1024-wide scalar-engine calls into a rolling fp16 pT buffer.
  Softmax without max-subtraction (exp(s/8 - 4) is fp16-safe); the
  denominator rides the AV matmul as a ones-augmented [V | 1] stationary
  (M=65); AV accumulates [65, 2 heads, 512 q] per block in two PSUM banks.
  Normalization: PE broadcast of the den row, vector reciprocal + multiply
  into fp16 attnT; proj accumulated from attnT in fp16, fp16 output.
"""

import numpy as np

B, S, E = 4, 2048, 1024
H, D = 16, 64
HLOC = 8          # heads per core
FEAT = HLOC * D   # 512 per-core q/k/v features
NCORES = 8
NP = HLOC // 2    # 4 head pairs
KC = S // 128     # 16 k-chunks
QC = S // 512     # 4 q-blocks per pair
CH = 512
NCH = S // CH     # 4 x-chunks
EC = E // 128     # 8 e-chunks
NSLOT = 15        # pT rolling slots (multiple of 5 = s1+s2 group period)

_CACHE = {}


def _build_program():
    import concourse.bass as bass
    import concourse.mybir as mybir
    from concourse import bacc
    from concourse.tile import TileContext
    from contextlib import ExitStack
    from collections import deque

    F32R = mybir.dt.float32r
    F32 = mybir.dt.float32
    F16 = mybir.dt.float16
    AF = mybir.ActivationFunctionType

    nc = bacc.Bacc("TRN2", target_bir_lowering=False, num_devices=NCORES)

    xt = nc.dram_tensor("xt", [E, S], F16, kind="ExternalInput")
    wq = nc.dram_tensor("wq", [E, FEAT], F16, kind="ExternalInput")
    wk = nc.dram_tensor("wk", [E, FEAT], F16, kind="ExternalInput")
    wv = nc.dram_tensor("wv", [E, FEAT], F16, kind="ExternalInput")
    wp = nc.dram_tensor("wp", [FEAT, E], F16, kind="ExternalInput")
    out = nc.dram_tensor("out", [S, E], F16, kind="ExternalOutput")

    xt_v = xt.ap().rearrange("(c p) s -> p c s", p=128)
    wq_v = wq.ap().rearrange("(c p) f -> p c f", p=128)
    wk_v = wk.ap().rearrange("(c p) f -> p c f", p=128)
    wv_v = wv.ap().rearrange("(c p) f -> p c f", p=128)
    wp_v = wp.ap().rearrange("(c p) n -> p c n", p=128)

    with TileContext(nc) as tc:
      with ExitStack() as es:
        from collections import deque as _dq

        pp = es.enter_context(tc.tile_pool(name="persist", bufs=1))
        qT = pp.tile([128, NP, S], F16)
        kT = pp.tile([128, NP, S], F16)
        v1 = pp.tile([128, KC, HLOC, D + 1], F16)
        attnT = pp.tile([128, NP, S], F16)
        pT = pp.tile([128, NSLOT, 512], F16)
        xt_sb = pp.tile([128, EC, S], F16)
        wq_sb = pp.tile([128, EC, FEAT], F16)
        wk_sb = pp.tile([128, EC, FEAT], F16)
        wv_sb = pp.tile([128, EC, FEAT], F16)
        wp_sb = pp.tile([128, NP, E], F16)
        ones_hi = pp.tile([128, 64], F32R)   # row 64 = ones (norm bcast lhsT)
        neg4 = pp.tile([128, 1], F32)

        pnm = es.enter_context(tc.tile_pool(name="pnorm", bufs=2))
        pout = es.enter_context(tc.tile_pool(name="pout", bufs=3))
        pstg = es.enter_context(tc.tile_pool(name="pstg", bufs=1, space="PSUM"))
        pso = es.enter_context(tc.tile_pool(name="pso", bufs=1, space="PSUM"))
        paux = es.enter_context(tc.tile_pool(name="paux", bufs=1, space="PSUM"))

        s1 = pstg.tile([128, 3, 512], F32)   # 3 banks: 1536-wide exp
        s2 = pstg.tile([128, 2, 512], F32)   # 2 banks: 1024-wide exp

        nc.gpsimd.memset(neg4[:], -4.0)

        # ---- DMA priority order: pair-0 weights + x chunk 0 first ----
        nc.sync.dma_start(wk_sb[:, :, 0:128], wk_v[:, :, 0:128])
        nc.sync.dma_start(wq_sb[:, :, 0:128], wq_v[:, :, 0:128])
        nc.sync.dma_start(xt_sb[:, 0:EC // 2, 0:CH], xt_v[:, 0:EC // 2, 0:CH])
        nc.sync.dma_start(xt_sb[:, EC // 2:, 0:CH], xt_v[:, EC // 2:, 0:CH])
        # constants: ones row + the ones column of [V | 1] (ACT const fill)
        nc.scalar.activation(ones_hi[64:65, :], wk_sb[64:65, 0, 0:64],
                             AF.Copy, bias=1.0, scale=0.0)
        nc.scalar.activation(
            v1[:, :, :, D],
            wk_sb[:, 0, 0:KC * HLOC].rearrange("p (a b) -> p a b", a=KC),
            AF.Copy, bias=1.0, scale=0.0)
        nc.sync.dma_start(wv_sb[:, 0:EC // 2], wv_v[:, 0:EC // 2])
        nc.sync.dma_start(wv_sb[:, EC // 2:], wv_v[:, EC // 2:])
        for ch in range(1, NCH):
            csl = slice(ch * CH, (ch + 1) * CH)
            nc.sync.dma_start(xt_sb[:, 0:EC // 2, csl],
                              xt_v[:, 0:EC // 2, csl])
            nc.sync.dma_start(xt_sb[:, EC // 2:, csl], xt_v[:, EC // 2:, csl])
        nc.sync.dma_start(wk_sb[:, :, 128:FEAT], wk_v[:, :, 128:FEAT])
        nc.sync.dma_start(wq_sb[:, :, 128:FEAT], wq_v[:, :, 128:FEAT])
        nc.sync.dma_start(wp_sb[:], wp_v)

        # ---------- aux-psum unit builders ----------
        # Units are ATOMIC: each emits its full matmul group + evacuation
        # before returning, so the single aux PSUM bank never has an open
        # accumulation group when another user allocates it (a mid-group
        # interleave would create a PE<->DVE FIFO deadlock).
        def qk_unit(dst, w_sb, fc, ch):
            """Q/K projection for one (pair, x-chunk)."""
            ps1 = paux.tile([128, 512], F32, tag="aux", name="ps1")
            csl = slice(ch * CH, (ch + 1) * CH)
            for ec in range(EC):
                nc.tensor.matmul(
                    ps1[:], w_sb[:, ec, fc * 128:(fc + 1) * 128],
                    xt_sb[:, ec, csl],
                    start=(ec == 0), stop=(ec == EC - 1))
            with nc.allow_low_precision(reason="fp16 attn"):
                nc.vector.tensor_copy(dst[:, fc, csl], ps1[:])

        def v_unit(kcg):
            """V projection for one 128-row k-chunk (all 8 heads)."""
            ps1 = paux.tile([128, 512], F32, tag="aux", name="ps1")
            ssl = slice(kcg * 128, (kcg + 1) * 128)
            for ec in range(EC):
                nc.tensor.matmul(ps1[:], xt_sb[:, ec, ssl], wv_sb[:, ec, :],
                                 start=(ec == 0), stop=(ec == EC - 1))
            with nc.allow_low_precision(reason="fp16 attn"):
                nc.vector.tensor_copy(
                    v1[:, kcg, :, 0:D],
                    ps1.rearrange("p (h d) -> p h d", h=HLOC))

        tail_mode = [False]
        tail_rot = [0]

        def proj_unit(sc, n2):
            """out[sc*128:+128, n2-half] = attnT.T @ wp (one 512-col half)."""
            ssl = slice(sc * 128, (sc + 1) * 128)
            nsl = slice(n2 * 512, (n2 + 1) * 512)
            if tail_mode[0]:
                # scores/exp are done: rotate through the idle stage banks
                # so tail proj units pipeline instead of serializing on the
                # single aux bank's evacuation.
                alts = [s1[:, 0, :], s1[:, 1, :], s1[:, 2, :],
                        s2[:, 0, :], s2[:, 1, :]]
                ps_p = alts[tail_rot[0] % len(alts)]
                tail_rot[0] += 1
            else:
                ps_p = paux.tile([128, 512], F32, tag="aux", name="ps_p")[:]
            for fc in range(NP):
                nc.tensor.matmul(ps_p, attnT[:, fc, ssl],
                                 wp_sb[:, fc, nsl],
                                 start=(fc == 0), stop=(fc == NP - 1))
            out_t = pout.tile([128, 512], F16, tag="out", name="out_t")
            with nc.allow_low_precision(reason="fp16 attn"):
                nc.vector.tensor_copy(out_t[:], ps_p)
            nc.sync.dma_start(out.ap()[ssl, nsl], out_t[:])

        work = _dq()

        def pull(n):
            while n > 0 and work:
                work.popleft()()
                n -= 1

        # ---------- attention emitter ----------
        cur_o = {}
        av_emitted = set()          # blocks whose kc==15 AV has been emitted
        g_slot = [0]                # next free pT slot (wraps by group)
        grp = {"tiles": [], "kind": 0}   # kind 0 -> s1 (3 slots), 1 -> s2 (2)
        av_levels = _dq()
        slot_of = {}                # (p, qc, kc) -> head-A pT slot
        pending_norms = _dq()       # (block, norm_fn)

        def av_pair(p, qc, kc, slot_a, slot_b):
            o = cur_o[(p, qc)]
            st, sp = (kc == 0), (kc == KC - 1)
            nc.tensor.matmul(o[0:D + 1, 0, :], v1[:, kc, 2 * p, :],
                             pT[:, slot_a, :], start=st, stop=sp)
            nc.tensor.matmul(o[0:D + 1, 1, :], v1[:, kc, 2 * p + 1, :],
                             pT[:, slot_b, :], start=st, stop=sp)
            if sp:
                av_emitted.add((p, qc))

        def drain_level():
            for p, qc, kc, head, slot in av_levels.popleft():
                if head == 0:
                    slot_of[(p, qc, kc)] = slot
                else:
                    av_pair(p, qc, kc, slot_of.pop((p, qc, kc)), slot)

        def flush_group():
            tiles = grp["tiles"]
            if not tiles:
                return
            st = s1 if grp["kind"] == 0 else s2
            n = len(tiles)
            if g_slot[0] + n > NSLOT:
                g_slot[0] = 0
            base = g_slot[0]
            g_slot[0] += n
            with nc.allow_low_precision(reason="fp16 attn"):
                nc.scalar.activation(pT[:, base:base + n, :], st[:, 0:n, :],
                                     AF.Exp, scale=0.125, bias=neg4[:])
            av_levels.append([(p, qc, kc, head, base + i)
                              for i, (p, qc, kc, head) in enumerate(tiles)])
            grp["tiles"] = []
            grp["kind"] ^= 1
            # emit AV lagged 2 exp-groups so the PE never waits on ACT
            while len(av_levels) > 2:
                drain_level()

        def emit_scores(p, qc, kc):
            """One k-chunk of scores for both heads of a pair: two slots."""
            ksl = slice(kc * 128, (kc + 1) * 128)
            qsl = slice(qc * 512, (qc + 1) * 512)
            for head in range(2):
                st = s1 if grp["kind"] == 0 else s2
                j = len(grp["tiles"])
                rows = slice(64 * head, 64 * head + 64)
                nc.tensor.matmul(st[:, j, :], kT[rows, p, ksl],
                                 qT[rows, p, qsl], start=True, stop=True)
                grp["tiles"].append((p, qc, kc, head))
                cap = 3 if grp["kind"] == 0 else 2
                if len(grp["tiles"]) == cap:
                    flush_group()

        def make_norm(p, qc, o):
            def norm():
                # Snapshot ps_o to SBUF in ONE copy: the psum accumulator is
                # then immediately free for the next block's AV matmuls; the
                # broadcast/reciprocal/normalize work off the snapshot.
                oc = pnm.tile([D + 1, 2, 512], F32R, tag="oc", name="o_copy")
                with nc.allow_low_precision(reason="fp16 attn"):
                    nc.vector.tensor_copy(oc[:], o[:])
                for h in range(2):
                    ps_b = paux.tile([128, 512], F32, tag="aux", name="ps_b")
                    nc.tensor.matmul(ps_b[0:64, :], ones_hi[64:65, :],
                                     oc[D:D + 1, h, :], start=True, stop=True)
                    r_sb = pnm.tile([64, 512], F32, tag=f"r{h}",
                                    name="r_sb")
                    nc.vector.reciprocal_approx_fast(out=r_sb[:],
                                                     in_=ps_b[0:64, :])
                    qsl = slice(qc * 512, (qc + 1) * 512)
                    with nc.allow_low_precision(reason="fp16 attn"):
                        nc.vector.tensor_mul(attnT[64 * h:64 * h + 64, p, qsl],
                                             oc[0:D, h, :], r_sb[:])
                if p == NP - 1:
                    for sc in range(4 * qc, 4 * qc + 4):
                        for n2 in range(2):
                            work.append(
                                lambda sc=sc, n2=n2: proj_unit(sc, n2))
            return norm

        def run_norm_front():
            b, fn = pending_norms[0]
            while b not in av_emitted:
                if not av_levels:
                    flush_group()
                else:
                    drain_level()
            pending_norms.popleft()
            fn()

        # ---------- pass 1: pair-0 q/k, all v, block (0,0) ----------
        cur_o[(0, 0)] = pso.tile([D + 1, 2, 512], F32, tag="o", name="ps_o")
        for ch in range(NCH):
            qk_unit(kT, wk_sb, 0, ch)
            qk_unit(qT, wq_sb, 0, ch)
            for sc2 in range(4):
                v_unit(4 * ch + sc2)
            for kcg in range(4 * ch, 4 * ch + 4):
                emit_scores(0, 0, kcg)
        pending_norms.append(((0, 0), make_norm(0, 0, cur_o[(0, 0)])))

        # ---------- pass 2: remaining blocks, qk pairs 1-3 via pulls ----
        for fc in range(1, NP):
            for ch in range(NCH):
                work.append(lambda f=fc, c=ch: qk_unit(kT, wk_sb, f, c))
                work.append(lambda f=fc, c=ch: qk_unit(qT, wq_sb, f, c))

        blocks = [(0, qc) for qc in range(1, QC)]
        blocks += [(p, qc) for p in range(1, NP) for qc in range(QC)]
        for p, qc in blocks:
            while pending_norms:
                run_norm_front()
            o = pso.tile([D + 1, 2, 512], F32, tag="o", name="ps_o")
            cur_o[(p, qc)] = o
            for kc in range(KC):
                emit_scores(p, qc, kc)
                if (p == NP - 1) or (kc % 3 == 0):
                    pull(1)
            pending_norms.append(((p, qc), make_norm(p, qc, o)))

        while pending_norms:
            run_norm_front()
        while av_levels:
            drain_level()
        tail_mode[0] = True
        pull(10 ** 9)

    nc.compile()
    return nc


def _prep_inputs(x, W_qkv, W_proj):
    """Build the 8 per-core input maps (host-side sharding/layout only)."""
    Wr = np.ascontiguousarray(W_qkv.reshape(E, 3, H, D))
    in_maps = []
    for c in range(NCORES):
        b, hg = c // 2, c % 2
        hsl = slice(hg * HLOC, (hg + 1) * HLOC)
        m = {
            "xt": np.ascontiguousarray(x[b].T).astype(np.float16),
            "wq": np.ascontiguousarray(
                Wr[:, 0, hsl, :].reshape(E, FEAT)).astype(np.float16),
            "wk": np.ascontiguousarray(
                Wr[:, 1, hsl, :].reshape(E, FEAT)).astype(np.float16),
            "wv": np.ascontiguousarray(
                Wr[:, 2, hsl, :].reshape(E, FEAT)).astype(np.float16),
            "wp": np.ascontiguousarray(
                W_proj[hg * FEAT:(hg + 1) * FEAT, :]).astype(np.float16),
        }
        in_maps.append(m)
    return in_maps


def _run_fallback(x, W_qkv, b_qkv, W_proj, b_proj):
    """Host-side reference path (only used when biases are nonzero)."""
    scale = 1.0 / np.sqrt(D)
    out = np.empty((B, S, E), dtype=np.float32)
    qkv = (x.reshape(B * S, E) @ W_qkv + b_qkv).reshape(B, S, 3, H, D)
    q, k, v = qkv[:, :, 0], qkv[:, :, 1], qkv[:, :, 2]
    for b in range(B):
        ob = np.empty((S, E), np.float32)
        for h in range(H):
            s = (q[b, :, h] @ k[b, :, h].T) * scale
            s -= s.max(axis=1, keepdims=True)
            p = np.exp(s)
            p /= p.sum(axis=1, keepdims=True)
            ob[:, h * D:(h + 1) * D] = p @ v[b, :, h]
        out[b] = ob @ W_proj + b_proj
    return out


def run(x, W_qkv, b_qkv, W_proj, b_proj, trace=False):
    from concourse.bass_utils import run_bass_kernel_spmd

    if bool(np.any(b_qkv)) or bool(np.any(b_proj)):
        return _run_fallback(x, W_qkv, b_qkv, W_proj, b_proj), None

    if "nc" not in _CACHE:
        _CACHE["nc"] = _build_program()
    nc = _CACHE["nc"]

    in_maps = _prep_inputs(x, W_qkv, W_proj)
    res = run_bass_kernel_spmd(nc, in_maps, core_ids=list(range(NCORES)),
                               trace=trace)
    out = np.empty((B, S, E), dtype=np.float32)
    for b in range(B):
        out[b] = (res.results[2 * b]["out"].astype(np.float32)
                  + res.results[2 * b + 1]["out"].astype(np.float32))
    return out, res


def kernel(x, W_qkv, b_qkv, W_proj, b_proj):
    out, _ = run(np.asarray(x), np.asarray(W_qkv), np.asarray(b_qkv),
                 np.asarray(W_proj), np.asarray(b_proj))
    return out
